# revision 4
# baseline (speedup 1.0000x reference)
"""Self-contained Trainium2 kernel for the NIC decoder module.

Strategy: data-parallel over the (sorted) batch, 16 rows per core on 8 cores.
Per core: mean over the 196 encoder positions (bf16 matmul with indicator
columns), LSTM init projections, 31 unrolled LSTM-with-gated-attention steps
(R2 layout: batch-transposed activations stationary on the PE, bf16 weights
streaming), then one batched fc over all (step, row) pairs with the vocab
projection streamed from HBM.  Ragged lengths are handled by masking h before
it is stored (dead rows are permanently dead, so feeding zeros forward is
exact for every surviving output).
"""

from contextlib import ExitStack

import numpy as np
import ml_dtypes

import concourse.bass as bass
import concourse.tile as tile
from concourse import bacc, mybir
from concourse.bass_utils import run_bass_kernel_spmd

F32 = mybir.dt.float32
BF16 = mybir.dt.bfloat16
SIG = mybir.ActivationFunctionType.Sigmoid
TANH = mybir.ActivationFunctionType.Tanh
BF = ml_dtypes.bfloat16

# Problem constants (hardcoded per the harness contract).
N, P, ENC, V, E, D, MAXLEN = 128, 196, 2048, 10000, 512, 512, 32
T = MAXLEN - 1            # 31 decode steps
NCORES = 8
NB = N // NCORES          # 16 batch rows per core
R = T * NB                # 496 (step, row) pairs per core, row index = t*16 + b

LAST_RESULTS = None       # set by kernel() for external inspection


def _build():
    nc = bacc.Bacc("TRN2", target_bir_lowering=False, debug=False,
                   num_devices=NCORES)

    def din(name, shape, dty):
        return nc.dram_tensor(name, shape, dty, kind="ExternalInput").ap()

    feat = din("feat", [NB, P, ENC], F32)
    eT = din("eT", [128, 4 * R], BF16)
    mask_col = din("mask_col", [NB, T], F32)
    mask_flat = din("mask_flat", [1, R], BF16)
    ind = din("ind", [128, NB * NB], BF16)
    id16 = din("id16", [16, 16], BF16)
    ones_r = din("ones_r", [1, 128], BF16)
    w_fb = din("w_fb", [128, 4 * ENC], BF16)
    w_a = din("w_a", [128, 16 * ENC], BF16)
    w_hh = din("w_hh", [128, 4 * ENC], BF16)
    w_e = din("w_e", [128, 4 * ENC], BF16)
    w_ith = din("w_ith", [128, 16 * D], BF16)
    w_itc = din("w_itc", [128, 16 * D], BF16)
    w_fc = din("w_fc", [128, 4 * V], BF16)
    b_fb = din("b_fb", [1, ENC], BF16)
    b_ihh = din("b_ihh", [1, 4 * D], BF16)
    b_ith = din("b_ith", [1, D], BF16)
    b_itc = din("b_itc", [1, D], BF16)
    b_fc = din("b_fc", [1, V], BF16)
    preds = nc.dram_tensor("preds", [R, V], F32, kind="ExternalOutput").ap()

    with tile.TileContext(nc) as tc, ExitStack() as ctx:
        cw = ctx.enter_context(tc.tile_pool(name="cw", bufs=1))
        ep_pool = ctx.enter_context(tc.tile_pool(name="epp", bufs=1))
        hT_pool = ctx.enter_context(tc.tile_pool(name="hTp", bufs=1))
        state = ctx.enter_context(tc.tile_pool(name="state", bufs=2))

        # ---- resident constants / weights -------------------------------
        s_ind = cw.tile([128, NB * NB], BF16)
        nc.sync.dma_start(s_ind[:], ind)
        s_id = cw.tile([16, 16], BF16)
        nc.sync.dma_start(s_id[:], id16)
        s_ones = cw.tile([1, 128], BF16)
        nc.sync.dma_start(s_ones[:], ones_r)
        s_maskc = cw.tile([NB, T], F32)
        nc.sync.dma_start(s_maskc[:], mask_col)
        s_maskf = cw.tile([1, R], BF16)
        nc.sync.dma_start(s_maskf[:], mask_flat)
        s_bfb = cw.tile([1, ENC], BF16)
        nc.sync.dma_start(s_bfb[:], b_fb)
        s_wfb = cw.tile([128, 4 * ENC], BF16)
        nc.sync.dma_start(s_wfb[:], w_fb)
        s_whh = cw.tile([128, 4 * ENC], BF16)
        nc.sync.dma_start(s_whh[:], w_hh)
        s_wa = cw.tile([128, 16 * ENC], BF16)
        nc.sync.dma_start(s_wa[:], w_a)

        e_dram = ep_pool.tile([R, ENC], BF16, space="DRAM")  # E_proj, t-major
        s_hT = hT_pool.tile([128, 4 * R], BF16)        # masked h, transposed
        s_h0T = hT_pool.tile([128, 64], BF16)
        s_mean = cw.tile([NB, ENC], F32)
        s_meanT = cw.tile([128, 256], BF16)

        # ================= phase A =======================================
        with tc.tile_pool(name="phA", bufs=1) as phA, \
             tc.tile_pool(name="psA", bufs=1, space="PSUM") as psA:

            # --- E_proj = embs @ W_e.T + (b_ih + b_hh) ------------------
            s_we = phA.tile([128, 4 * ENC], BF16)
            nc.sync.dma_start(s_we[:], w_e)
            s_eT = phA.tile([128, 4 * R], BF16)
            nc.sync.dma_start(s_eT[:], eT)
            s_bihh = phA.tile([1, 4 * D], BF16)
            nc.sync.dma_start(s_bihh[:], b_ihh)

            ps_mean = psA.tile([NB, ENC], F32, tag="mean")
            for n4 in range(4):
                ns = slice(n4 * 512, (n4 + 1) * 512)
                for m in range(4):
                    msz = 128 if m < 3 else R - 3 * 128
                    ps_ep = psA.tile([128, 512], F32, tag="ep", name=f"ep{n4}_{m}")
                    for k in range(4):
                        nc.tensor.matmul(
                            ps_ep[:msz, :],
                            s_eT[:, k * R + m * 128: k * R + m * 128 + msz],
                            s_we[:, k * ENC + n4 * 512: k * ENC + (n4 + 1) * 512],
                            start=(k == 0), stop=False)
                    nc.tensor.matmul(ps_ep[:msz, :], s_ones[0:1, 0:msz],
                                     s_bihh[0:1, ns], start=False, stop=True)
                    epst = phA.tile([128, 512], BF16, tag="epst", bufs=4,
                                    name=f"epst{n4}_{m}")
                    nc.vector.tensor_copy(epst[:msz, :], ps_ep[:msz, :])
                    nc.sync.dma_start(e_dram[m * 128: m * 128 + msz, ns],
                                      epst[:msz, :])

            # --- mean over the 196 positions ----------------------------
            first = True
            for b in range(NB):
                for c, (p0, pc) in enumerate(((0, 128), (128, P - 128))):
                    ft = phA.tile([128, ENC], F32, tag="feat", name=f"ft{b}_{c}")
                    nc.sync.dma_start(ft[:pc, :], feat[b, p0:p0 + pc, :])
                    fb_ = phA.tile([128, ENC], BF16, tag="featbf", name=f"fb{b}_{c}")
                    if (b + c) % 2 == 0:
                        nc.scalar.copy(fb_[:pc, :], ft[:pc, :])
                    else:
                        nc.vector.tensor_copy(fb_[:pc, :], ft[:pc, :])
                    last = (b == NB - 1 and c == 1)
                    for n4 in range(4):
                        ns = slice(n4 * 512, (n4 + 1) * 512)
                        nc.tensor.matmul(ps_mean[:, ns],
                                         s_ind[0:pc, b * 16:(b + 1) * 16],
                                         fb_[0:pc, ns],
                                         start=first, stop=last)
                    first = False
            nc.scalar.mul(s_mean[:], ps_mean[:], 1.0 / float(P))
            s_meanbf = phA.tile([NB, ENC], BF16)
            nc.vector.tensor_copy(s_meanbf[:], s_mean[:])

            # meanT (for the init projections)
            ps_mt = psA.tile([128, 256], BF16, tag="mean", name="ps_mt")
            for e in range(16):
                nc.tensor.transpose(ps_mt[:, e * 16:(e + 1) * 16],
                                    s_meanbf[:, e * 128:(e + 1) * 128], s_id[:])
            nc.vector.tensor_copy(s_meanT[:], ps_mt[:])

            # --- init projections h0 / c0 -------------------------------
            s_bith = phA.tile([1, D], BF16)
            nc.sync.dma_start(s_bith[:], b_ith)
            s_bitc = phA.tile([1, D], BF16)
            nc.sync.dma_start(s_bitc[:], b_itc)

            ps_h0 = psA.tile([NB, D], F32, tag="ep", name="ps_h0")
            ps_c0 = psA.tile([NB, D], F32, tag="ep", name="ps_c0")
            for which, (wsrc, bsrc, pst) in enumerate(
                    ((w_ith, s_bith, ps_h0), (w_itc, s_bitc, ps_c0))):
                for k in range(16):
                    wk = phA.tile([128, D], BF16, tag="wi", bufs=4,
                                  name=f"wi{which}_{k}")
                    nc.sync.dma_start(wk[:], wsrc[:, k * D:(k + 1) * D])
                    nc.tensor.matmul(pst[:], s_meanT[:, k * 16:(k + 1) * 16],
                                     wk[:], start=(k == 0), stop=False)
                nc.tensor.matmul(pst[:], s_ones[0:1, 0:16], bsrc[0:1, :],
                                 start=False, stop=True)

            c0 = state.tile([NB, D], F32, tag="c")
            nc.vector.tensor_copy(c0[:], ps_c0[:])
            h0bf = phA.tile([NB, D], BF16)
            nc.vector.tensor_copy(h0bf[:], ps_h0[:])
            ps_h0t = psA.tile([128, 64], BF16, tag="mean", name="ps_h0t")
            for k in range(4):
                nc.tensor.transpose(ps_h0t[:, k * 16:(k + 1) * 16],
                                    h0bf[:, k * 128:(k + 1) * 128], s_id[:])
            nc.vector.tensor_copy(s_h0T[:], ps_h0t[:])

        # ================= recurrence ====================================
        hT_view = s_hT.rearrange("p (k r) -> p k r", k=4)
        c_prev = c0
        with tc.tile_pool(name="tmp", bufs=1) as tmp, \
             tc.tile_pool(name="tmp2", bufs=2) as tmp2, \
             tc.tile_pool(name="psR", bufs=1, space="PSUM") as psR:
            for t in range(T):
                if t == 0:
                    def hT_k(k):
                        return s_h0T[:, k * 16:(k + 1) * 16]
                else:
                    def hT_k(k, _t=t):
                        return s_hT[:, k * R + (_t - 1) * 16: k * R + _t * 16]

                gate_ps = psR.tile([NB, ENC], F32, tag="gate", name=f"gate{t}")
                for k in range(4):
                    for n4 in range(4):
                        ns = slice(n4 * 512, (n4 + 1) * 512)
                        nc.tensor.matmul(gate_ps[:, ns], hT_k(k),
                                         s_wfb[:, k * ENC + n4 * 512:
                                               k * ENC + (n4 + 1) * 512],
                                         start=(k == 0), stop=False)
                for n4 in range(4):
                    ns = slice(n4 * 512, (n4 + 1) * 512)
                    nc.tensor.matmul(gate_ps[:, ns], s_ones[0:1, 0:16],
                                     s_bfb[0:1, ns], start=False, stop=True)

                awe = []
                for n4 in range(4):
                    ns = slice(n4 * 512, (n4 + 1) * 512)
                    gs = tmp2.tile([NB, 512], F32, tag="gs", name=f"gs{t}_{n4}")
                    nc.scalar.activation(gs[:], gate_ps[:, ns], SIG)
                    ab = tmp2.tile([NB, 512], BF16, tag="ab", name=f"ab{t}_{n4}")
                    nc.vector.tensor_mul(ab[:], gs[:], s_mean[:, ns])
                    awe.append(ab)

                ptrA = psR.tile([128, 256], BF16, tag="gate", name=f"ptrA{t}")
                for n4 in range(4):
                    for j in range(4):
                        e = n4 * 4 + j
                        nc.tensor.transpose(ptrA[:, e * 16:(e + 1) * 16],
                                            awe[n4][:, j * 128:(j + 1) * 128],
                                            s_id[:])
                aweT = tmp2.tile([128, 256], BF16, tag="awT", name=f"awT{t}")
                nc.vector.tensor_copy(aweT[:], ptrA[:])

                g_ps = psR.tile([NB, ENC], F32, tag="g", name=f"g{t}")
                for k in range(4):
                    for n4 in range(4):
                        ns = slice(n4 * 512, (n4 + 1) * 512)
                        nc.tensor.matmul(g_ps[:, ns], hT_k(k),
                                         s_whh[:, k * ENC + n4 * 512:
                                               k * ENC + (n4 + 1) * 512],
                                         start=(k == 0), stop=False)
                for k in range(16):
                    for n4 in range(4):
                        ns = slice(n4 * 512, (n4 + 1) * 512)
                        nc.tensor.matmul(g_ps[:, ns],
                                         aweT[:, k * 16:(k + 1) * 16],
                                         s_wa[:, k * ENC + n4 * 512:
                                               k * ENC + (n4 + 1) * 512],
                                         start=False, stop=(k == 15))

                ep_t = tmp2.tile([NB, ENC], BF16, tag="ept", bufs=3,
                                 name=f"ept{t}")
                nc.sync.dma_start(ep_t[:], e_dram[t * 16:(t + 1) * 16, :])
                g_sb = tmp.tile([NB, ENC], F32, tag="gsb", name=f"gsb{t}")
                nc.vector.tensor_add(g_sb[:], g_ps[:], ep_t[:])

                act = tmp.tile([NB, ENC], F32, tag="act", name=f"act{t}")
                nc.scalar.activation(act[:, 0:1024], g_sb[:, 0:1024], SIG)
                nc.scalar.activation(act[:, 1024:1536], g_sb[:, 1024:1536], TANH)
                nc.scalar.activation(act[:, 1536:2048], g_sb[:, 1536:2048], SIG)

                m1 = tmp2.tile([NB, D], F32, tag="m1", name=f"m1_{t}")
                nc.vector.tensor_mul(m1[:], act[:, 512:1024], c_prev[:])
                m2 = tmp2.tile([NB, D], F32, tag="m2", name=f"m2_{t}")
                nc.vector.tensor_mul(m2[:], act[:, 0:512], act[:, 1024:1536])
                c_new = state.tile([NB, D], F32, tag="c", name=f"c{t}")
                nc.vector.tensor_add(c_new[:], m1[:], m2[:])
                tct = tmp2.tile([NB, D], F32, tag="tc", name=f"tc{t}")
                nc.scalar.activation(tct[:], c_new[:], TANH)
                h_new = tmp2.tile([NB, D], F32, tag="hn", name=f"hn{t}")
                nc.vector.tensor_mul(h_new[:], act[:, 1536:2048], tct[:])
                hm = tmp2.tile([NB, D], BF16, tag="hm", name=f"hm{t}")
                nc.vector.tensor_scalar_mul(hm[:], h_new[:],
                                            s_maskc[:, t:t + 1])

                ptrH = psR.tile([128, 64], BF16, tag="gate", name=f"ptrH{t}")
                for k in range(4):
                    nc.tensor.transpose(ptrH[:, k * 16:(k + 1) * 16],
                                        hm[:, k * 128:(k + 1) * 128], s_id[:])
                nc.vector.tensor_copy(
                    hT_view[:, :, t * 16:(t + 1) * 16],
                    ptrH[:].rearrange("p (k r) -> p k r", k=4))
                c_prev = c_new

        # ================= fc ============================================
        with tc.tile_pool(name="fcw", bufs=1) as fcw, \
             tc.tile_pool(name="psF", bufs=1, space="PSUM") as psF:
            NCH = 500
            for n20 in range(V // NCH):
                ns = slice(n20 * NCH, (n20 + 1) * NCH)
                wts = []
                for k in range(4):
                    wt = fcw.tile([128, NCH], BF16, tag="wfc", bufs=8,
                                  name=f"wfc{n20}_{k}")
                    nc.sync.dma_start(wt[:], w_fc[:, k * V + n20 * NCH:
                                                  k * V + (n20 + 1) * NCH])
                    wts.append(wt)
                bt = fcw.tile([1, NCH], BF16, tag="bfc", bufs=2,
                              name=f"bfc{n20}")
                nc.sync.dma_start(bt[:], b_fc[0:1, ns])
                for m in range(4):
                    msz = 128 if m < 3 else R - 3 * 128
                    pfc = psF.tile([128, NCH], F32, tag="fc", bufs=4,
                                   name=f"pfc{n20}_{m}")
                    for k in range(4):
                        nc.tensor.matmul(
                            pfc[:msz, :],
                            s_hT[:, k * R + m * 128: k * R + m * 128 + msz],
                            wts[k][:], start=(k == 0), stop=False)
                    nc.tensor.matmul(pfc[:msz, :],
                                     s_maskf[0:1, m * 128: m * 128 + msz],
                                     bt[0:1, :], start=False, stop=True)
                    st = fcw.tile([128, NCH], F32, tag="stage", bufs=4,
                                  name=f"st{n20}_{m}")
                    if m % 2 == 0:
                        nc.vector.tensor_copy(st[:msz, :], pfc[:msz, :])
                    else:
                        nc.scalar.copy(st[:msz, :], pfc[:msz, :])
                    nc.sync.dma_start(preds[m * 128: m * 128 + msz, ns],
                                      st[:msz, :])
    nc.compile()
    return nc


_CACHED = None


def _get_nc():
    global _CACHED
    if _CACHED is None:
        _CACHED = _build()
    return _CACHED


def _pack_rhs(wT):
    """(K, N) f32 -> (128, (K//128)*N) bf16, K-tile-major free layout."""
    K_, N_ = wT.shape
    kt = K_ // 128
    return np.ascontiguousarray(
        wT.reshape(kt, 128, N_).transpose(1, 0, 2).reshape(128, kt * N_)
    ).astype(BF)


def kernel(enc_feature, enc_capt, capt_lengths, emb, W_ih, b_ih, W_hh, b_hh,
           W_inith, b_inith, W_initc, b_initc, W_fbeta, b_fbeta, W_fc, b_fc):
    global LAST_RESULTS
    enc_feature = np.asarray(enc_feature, dtype=np.float32)
    emb = np.asarray(emb, dtype=np.float32)

    cl = np.asarray(capt_lengths).reshape(N).astype(np.int64)
    sort_ind = np.argsort(-cl, kind="stable")
    cl_s = cl[sort_ind]
    capt_s = np.asarray(enc_capt)[sort_ind]
    dec_len = cl_s - 1
    mask = (np.arange(T)[None, :] < dec_len[:, None]).astype(np.float32)

    feat_s = enc_feature.reshape(N, P, ENC)[sort_ind]
    embs = emb[capt_s[:, :T].astype(np.int64)]          # (N, T, E) f32

    # shared (per-core identical) tensors
    shared = {
        "ind": np.ascontiguousarray(
            np.eye(NB, dtype=np.float32)[None].repeat(128, 0)
            .transpose(0, 2, 1).reshape(128, NB * NB)).astype(BF),
        "id16": np.eye(16, dtype=np.float32).astype(BF),
        "ones_r": np.ones((1, 128), dtype=np.float32).astype(BF),
        "w_fb": _pack_rhs(np.ascontiguousarray(np.asarray(W_fbeta).T)),
        "w_a": _pack_rhs(np.ascontiguousarray(np.asarray(W_ih)[:, E:].T)),
        "w_hh": _pack_rhs(np.ascontiguousarray(np.asarray(W_hh).T)),
        "w_e": _pack_rhs(np.ascontiguousarray(np.asarray(W_ih)[:, :E].T)),
        "w_ith": _pack_rhs(np.ascontiguousarray(np.asarray(W_inith).T)),
        "w_itc": _pack_rhs(np.ascontiguousarray(np.asarray(W_initc).T)),
        "w_fc": _pack_rhs(np.ascontiguousarray(np.asarray(W_fc).T)),
        "b_fb": np.asarray(b_fbeta).reshape(1, ENC).astype(BF),
        "b_ihh": (np.asarray(b_ih) + np.asarray(b_hh)).reshape(1, 4 * D).astype(BF),
        "b_ith": np.asarray(b_inith).reshape(1, D).astype(BF),
        "b_itc": np.asarray(b_initc).reshape(1, D).astype(BF),
        "b_fc": np.asarray(b_fc).reshape(1, V).astype(BF),
    }

    in_maps = []
    for c in range(NCORES):
        rows = slice(c * NB, (c + 1) * NB)
        ef = embs[rows].transpose(1, 0, 2).reshape(R, E)      # t-major
        eTc = np.ascontiguousarray(
            ef.T.reshape(4, 128, R).transpose(1, 0, 2).reshape(128, 4 * R)
        ).astype(BF)
        m_c = mask[rows]                                       # (16, 31)
        im = dict(shared)
        im["feat"] = np.ascontiguousarray(feat_s[rows])
        im["eT"] = eTc
        im["mask_col"] = np.ascontiguousarray(m_c)
        im["mask_flat"] = np.ascontiguousarray(
            m_c.T.reshape(1, R)).astype(BF)
        in_maps.append(im)

    nc = _get_nc()
    res = run_bass_kernel_spmd(nc, in_maps, core_ids=list(range(NCORES)))
    LAST_RESULTS = res

    predictions = np.concatenate(
        [res.results[c]["preds"].reshape(T, NB, V).transpose(1, 0, 2)
         for c in range(NCORES)], axis=0)

    alphas = np.broadcast_to(
        (mask / np.float32(P))[:, :, None], (N, T, P)).copy()

    return (predictions,
            capt_s.astype(np.int32),
            dec_len.astype(np.int32),
            alphas.astype(np.float32),
            sort_ind.astype(np.int32))
